# revision 13
# baseline (speedup 1.0000x reference)
"""NeuralWDRC Trainium2 kernel: 8-core data-parallel (2 batch samples/core).

v2 redesign (per core):
  1) MLP (collapsed, bf16): ratio_t = clip(softplus(p2)+1, 1, 20); p2 via the
     prelu(v)=c*v+d*|v| rewrite with host-folded stacked weights and a
     lambda-encoded extra row so K=65,N=1 f32 matmuls emit p2 with t on
     partitions.  x staged bf16 (SWDGE cast-DMA), PE transposes bf16, psum->
     sbuf copies via DMA, matmul weights bf16, ACT Abs epilogues.
  2) Interp: ratio[T] -> ratio_i[N] as bf16 K=3 matmuls into PSUM groups;
     rinv = reciprocal_approx_fast directly from PSUM (no copy pass).
  3) Compress x2 with the algebra  gain = clip(1 + q*relu(1 - thr/env)):
       b = (thr/|x|) via ACT Abs_reciprocal_sqrt + Square   (2 ACT passes)
       h = min(b-1, 0) = -relu(1 - thr/env)                 (DVE TS, bf16 4x)
       t = Qbar*h where Qbar=-q from rinv                   (TT bf16)
       g = clip via one fused TS; IIR smoothing via DVE tensor_tensor_scan
     with 64-sample halo warmup (0.9^64~1.2e-3) and exact stream-start fixup.
     The 0.1 residual-mix and alpha=0.1 scan scale are folded into Qbar/clip
     and the final normalization (v = 10*out_unnorm cancels there).
  4) Global abs-max normalization: per-core maxima -> AllGather(8) -> scale.
"""

import math
import numpy as np

import concourse.bass as bass
import concourse.bacc as bacc
import concourse.mybir as mybir
import concourse.tile as tile
from concourse.bass_utils import run_bass_kernel_spmd
from concourse import bass_isa

F32 = mybir.dt.float32
BF16 = mybir.dt.bfloat16
AF = mybir.ActivationFunctionType
OP = mybir.AluOpType

NCORES = 8
S = 2                 # samples per core
T = 4000              # MLP timesteps per sample
TB = S * T            # 8000
NSAMP = 320000        # audio samples per batch item
HOP = 80
GRU_H, H1, H2 = 256, 128, 64

P = 125               # audio partitions (125 * 2560 = 320000)
CH = 2560             # audio cols per partition
HH = CH // 2          # 1280: tail-pipelined column half
W = 64                # scan halo (warmup) cols
AH = W + HH           # 1344
CHK = 500             # MLP t-chunk
NCHK = TB // CHK      # 16
TT = 125              # t subtile (transpose / interp)
LAM = 2.0 ** -10

_compiled = {}


def _prep_weights(W1, b1, a1, W2, b2, a2, W3, b3):
    """Host-side weight composition (float64 for accuracy)."""
    W1 = W1.astype(np.float64); W2 = W2.astype(np.float64)
    w3 = W3.astype(np.float64)[2]          # only the ratio output row
    b1 = b1.astype(np.float64); b2 = b2.astype(np.float64)
    b3r = float(np.asarray(b3, np.float64)[2])
    a1 = float(a1); a2 = float(a2)
    c1, d1 = (1 + a1) / 2, (1 - a1) / 2
    c2, d2 = (1 + a2) / 2, (1 - a2) / 2

    A2 = c1 * (W2 @ W1)                    # [64, 256]
    B2 = d1 * W2                           # [64, 128]
    beta2 = b2 + c1 * (W2 @ b1)            # [64]

    a3 = c2 * (A2.T @ w3)                  # [256]
    b3v = c2 * (B2.T @ w3)                 # [128]
    c3v = d2 * w3                          # [64]
    gamma = c2 * float(w3 @ beta2) + b3r

    A2x = np.concatenate([A2, LAM * a3[None, :]], 0)    # [65, 256]
    B2x = np.concatenate([B2, LAM * b3v[None, :]], 0)   # [65, 128]
    beta2x = np.concatenate([beta2, [1.0]])             # [65]
    r3 = np.concatenate([c3v, [1.0 / LAM]])             # [65]
    spb = gamma - 1.0 / LAM                              # scalar

    W1T = W1.T                                           # [256, 128]
    out = {
        "w1t0": W1T[:128], "w1t1": W1T[128:],
        "a2xt0": A2x.T[:128], "a2xt1": A2x.T[128:],      # [128, 65]
        "b2xt": B2x.T,                                   # [128, 65]
        "r3": r3[:, None],                               # [65, 1]
        "bias1": b1[:, None],                            # [128, 1]
        "bias2": beta2x[:, None],                        # [65, 1]
        "spbias": np.full((P, 1), spb),                  # [125, 1]
    }
    return {k: np.ascontiguousarray(v, np.float32) for k, v in out.items()}


def _interp_m3():
    """[3, 80] weights: ratio_i[80t+k] = sum_j M3[j,k] * ratio[t-1+j]."""
    m = np.zeros((3, HOP), np.float64)
    for k in range(HOP):
        f = (k + 0.5) / HOP - 0.5
        if k < HOP // 2:
            m[0, k] = -f
            m[1, k] = 1.0 + f
        else:
            m[1, k] = 1.0 - f
            m[2, k] = f
    return np.ascontiguousarray(m, np.float32)


def _build_nc(sim=False):
    nc = bacc.Bacc("TRN2", target_bir_lowering=False, debug=False,
                   num_devices=NCORES)
    gru = nc.dram_tensor("gru", [TB, GRU_H], F32, kind="ExternalInput")
    enh = nc.dram_tensor("enh", [S, NSAMP], F32, kind="ExternalInput")
    noisy = nc.dram_tensor("noisy", [S, NSAMP], F32, kind="ExternalInput")
    wnames = ["w1t0", "w1t1", "a2xt0", "a2xt1", "b2xt", "r3",
              "bias1", "bias2", "spbias", "m3d", "ident"]
    wshapes = {"w1t0": [128, 128], "w1t1": [128, 128],
               "a2xt0": [128, 65], "a2xt1": [128, 65], "b2xt": [128, 65],
               "r3": [65, 1], "bias1": [128, 1], "bias2": [65, 1],
               "spbias": [P, 1], "m3d": [35, HOP], "ident": [128, 128]}
    wdram = {n: nc.dram_tensor(n, wshapes[n], F32, kind="ExternalInput")
             for n in wnames}
    out = nc.dram_tensor("out", [S, NSAMP], F32, kind="ExternalOutput")
    cc_in = nc.dram_tensor("cc_in", [2], F32)
    cc_out = nc.dram_tensor("cc_out", [2 * NCORES], F32, addr_space="Shared")

    ncc = S * T // TT  # 64 p2 columns
    rows = ncc // S    # 32 ratT rows per sample
    with tile.TileContext(nc) as tc:
        with (
            tc.tile_pool(name="wpool", bufs=1) as wpool,
            tc.tile_pool(name="mlp", bufs=2) as mlp,
            tc.tile_pool(name="small", bufs=1) as small,
            tc.tile_pool(name="scr", bufs=1) as scr,
            tc.tile_pool(name="aud", bufs=1) as aud,
            tc.tile_pool(name="ps", bufs=2, space="PSUM") as ps,
            tc.tile_pool(name="ps1", bufs=1, space="PSUM") as ps1,
            tc.tile_pool(name="ps2", bufs=2, space="PSUM") as ps2,
        ):
            # ---- resident weights (f32 staged, bf16 copies for PE) ----
            wsb = {}
            for n in wnames:
                t_ = wpool.tile(wshapes[n], F32, tag=n)
                nc.sync.dma_start(t_[:], wdram[n][:])
                wsb[n] = t_
            identb = wpool.tile([128, 128], BF16, tag="identb")
            nc.vector.tensor_copy(identb[:], wsb["ident"][:])
            wsr = {}
            for n in ("w1t0", "w1t1", "a2xt0", "a2xt1", "b2xt", "m3d"):
                t_ = wpool.tile(wshapes[n], BF16, tag=n + "b")
                nc.vector.tensor_copy(t_[:], wsb[n][:])
                wsr[n] = t_

            # scan decay tile (f32 so 0.9 is exact; fp32 scan state)
            d0 = wpool.tile([P, AH], F32, tag="d0")
            nc.gpsimd.memset(d0[:], 0.9)

            p2ps = ps1.tile([P, ncc], F32, tag="p2")  # [125, 64]

            def pe_touch(ap):
                # Absorb one cross-engine dep into a trivial PE matmul so the
                # following self-loading matmuls carry <=1 sync wait.
                if mybir.dt.size(ap.dtype) == 2:
                    a = ap[0:1, 0:2].bitcast(F32)
                else:
                    a = ap[0:1, 0:1].bitcast(F32)
                dmy = ps1.tile([1, 2], F32, tag="dmy")
                nc.tensor.matmul(dmy[0:1, 0:1], a, a, start=True, stop=True)

            epsb = small.tile([P, 1], F32, tag="epsb")
            nc.gpsimd.memset(epsb[:], 1e-8)
            vm = small.tile([P, 2 * S], F32, tag="vm")   # per-half |v| maxima
            em = small.tile([P, S], F32, tag="em")       # per-sample |enh| max
            vout = [None, None]
            sh3 = wpool.tile([35, T + 34], BF16, tag="sh3")

            # ---- audio loads (bf16 cast-DMA) + rinv-independent pre-passes
            # SWDGE order: gru q0,q1 -> audio s0 -> gru q2,q3 -> audio s1 ->
            # gru q4..q7 so the MLP starts immediately.
            CW = (CHK // TT) * GRU_H  # 1024 cols per chunk
            xa = wpool.tile([TT, NCHK * CW], BF16, tag="xa")

            def gru_load(q):
                src_ = gru[q * (TB // 8):(q + 1) * (TB // 8), :].rearrange(
                    "(c j p) h -> p c j h", p=TT, j=CHK // TT)
                dst = xa[:, q * 2 * CW:(q + 1) * 2 * CW].rearrange(
                    "p (c j h) -> p c j h", j=CHK // TT, h=GRU_H)
                nc.gpsimd.dma_start(dst, src_)

            audio_sr = []
            for s in range(S):
                enh_t = aud.tile([P, CH], BF16, tag=f"enh{s}")
                noi_t = aud.tile([P, CH], BF16, tag=f"res{s}")
                audio_sr.append((enh_t, noi_t))
            gru_load(0)
            gru_load(1)
            for s in range(S):
                enh_t, noi_t = audio_sr[s]
                nc.gpsimd.dma_start(enh_t[:],
                                    enh[s].rearrange("(p n) -> p n", p=P))
                nc.gpsimd.dma_start(noi_t[:],
                                    noisy[s].rearrange("(p n) -> p n", p=P))
                # res = noisy - enh (in place)
                nc.vector.tensor_tensor(noi_t[:], noi_t[:], enh_t[:],
                                        op=OP.subtract)
                if s == 0:
                    gru_load(2)
                    gru_load(3)
            for q in range(4, 8):
                gru_load(q)

            hh_sr = [None, None]

            def audio_pre(s):
                enh_t, res_t = audio_sr[s]
                # env = |x|+eps (ACT Abs), rec = 1/env (DVE), h' = min(rec-1/thr, 0)
                # (= -relu(1-thr/env)/thr; thr refolded into Qbar)
                env = scr.tile([P, CH], F32, tag="env")
                rec = scr.tile([P, CH], F32, tag="rec")
                h_e = aud.tile([P, CH], BF16, tag=f"he{s}")
                nc.scalar.activation(env[:], enh_t[:], AF.Abs, bias=epsb[:, 0:1])
                nc.vector.reciprocal_approx_fast(out=rec[:], in_=env[:])
                nc.vector.tensor_scalar(h_e[:], rec[:], 1.0 / 0.3, 0.0,
                                        op0=OP.subtract, op1=OP.min)
                env2 = scr.tile([P, CH], F32, tag="env")
                rec2 = scr.tile([P, CH], F32, tag="rec")
                h_r = aud.tile([P, CH], BF16, tag=f"hr{s}")
                nc.scalar.activation(env2[:], res_t[:], AF.Abs, bias=epsb[:, 0:1])
                nc.vector.reciprocal_approx_fast(out=rec2[:], in_=env2[:])
                nc.vector.tensor_scalar(h_r[:], rec2[:], 1.0 / 0.1, 0.0,
                                        op0=OP.subtract, op1=OP.min)
                # emax = max|enh| (bf16 audio is the v source, consistent)
                nc.vector.tensor_reduce(em[:, s:s + 1], enh_t[:], op=OP.max,
                                        axis=mybir.AxisListType.X,
                                        apply_absolute_value=True)
                hh_sr[s] = (h_e, h_r)

            audio_pre(0)

            post_ctx = {}

            def post_stage0(s):
                b = rows * s
                # ---- ratio chain on [125, 32] for this sample ----
                rat = small.tile([P, rows], F32, tag=f"rat{s}")
                nc.vector.tensor_copy(rat[:], p2ps[:, s * rows:(s + 1) * rows])
                # softplus(x) = ln(1+exp(x)); table set natural_log_exp
                nc.scalar.activation(rat[:], rat[:], AF.Exp,
                                     bias=wsb["spbias"][:])
                nc.scalar.activation(rat[:], rat[:], AF.Ln, bias=1.0)
                nc.vector.tensor_scalar(rat[:], rat[:], 1.0, 20.0,
                                        op0=OP.add, op1=OP.min)
                ratT_ps = ps2.tile([rows, P], F32, tag="rt")
                pe_touch(rat)
                nc.tensor.transpose(ratT_ps[:], rat[:], wsb["ident"][:P, :P])
                ratT = small.tile([rows, P], BF16, tag=f"ratT{s}")
                nc.scalar.copy(ratT[:], ratT_ps[:])

                # shifted-ratio rows: row b+j, col i = ratio_s[i+j-1] (clipped)
                rT = ratT[:]
                r3d = lambda ap: ap.rearrange("p (r q) -> p r q", q=P)
                nc.sync.dma_start(r3d(sh3[b:b + 1, 1:T + 1]), rT)
                nc.sync.dma_start(sh3[b:b + 1, 0:1], rT[0:1, 0:1])
                nc.sync.dma_start(r3d(sh3[b + 1:b + 2, 0:T]), rT)
                nc.sync.dma_start(sh3[b + 2:b + 3, 0:124], rT[0:1, 1:P])
                nc.sync.dma_start(
                    r3d(sh3[b + 2:b + 3, 124:124 + 31 * P]), rT[1:rows, :])
                nc.sync.dma_start(sh3[b + 2:b + 3, T - 1:T],
                                  rT[rows - 1:rows, P - 1:P])

            def post_stage1(s):
                b = rows * s
                # ---- interp (bf16 matmuls) -> rinv via recip from PSUM ----
                pe_touch(sh3[0:1, 0:2] if s == 0 else sh3[0:1, 2:4])
                rinv = scr.tile([P, CH], F32, tag="rinv")
                ngrp = (rows + 5) // 6
                for g in range(ngrp):
                    taus = list(range(g * 6, min((g + 1) * 6, rows)))
                    rips = ps2.tile([P, 480], F32, tag="rt")
                    for ti, tau in enumerate(taus):
                        lhsT = sh3[b:b + 3, tau:tau + 32 * P:32]  # [3, 125]
                        nc.tensor.matmul(rips[:, ti * HOP:(ti + 1) * HOP],
                                         lhsT, wsr["m3d"][b:b + 3, :],
                                         start=True, stop=True)
                    nwid = len(taus) * HOP
                    nc.vector.reciprocal_approx_fast(
                        out=rinv[:, g * 480:g * 480 + nwid],
                        in_=rips[:, :nwid])

                # Qbar_e = 0.3(1-rinv); Qbar_r = 0.01-0.02*rinv (0.1 mix and
                # the 1/thr of h' folded in)
                qe = aud.tile([P, CH], BF16, tag="qe")
                qr = aud.tile([P, CH], BF16, tag="qr")
                if s == 0:
                    for h in range(2):
                        hs = slice(HH * h, HH * (h + 1))
                        nc.vector.tensor_scalar(qe[:, hs], rinv[:, hs],
                                                -0.3, 0.3,
                                                op0=OP.mult, op1=OP.add)
                        nc.vector.tensor_scalar(qr[:, hs], rinv[:, hs],
                                                -0.02, 0.01,
                                                op0=OP.mult, op1=OP.add)
                else:
                    nc.scalar.activation(qe[:], rinv[:], AF.Copy,
                                         bias=0.3, scale=-0.3)
                    nc.scalar.activation(qr[:], rinv[:], AF.Copy,
                                         bias=0.01, scale=-0.02)
                ge = aud.tile([P, 2 * AH], BF16, tag="ge")
                gr = aud.tile([P, 2 * AH], BF16, tag="gr")
                se = aud.tile([P, 2 * AH], BF16, tag="se")
                sr_ = aud.tile([P, 2 * AH], BF16, tag="sr")
                v = aud.tile([P, CH], BF16, tag=f"v{s}")
                vout[s] = v
                post_ctx[s] = (qe, qr, ge, gr, se, sr_, v)

            def post_gain_half(s, h):
                qe, qr, ge, gr, se, sr_, v = post_ctx[s]
                h_e, h_r = hh_sr[s]
                hs = slice(HH * h, HH * (h + 1))
                gsl = slice(AH * h + W, AH * (h + 1))
                te = scr.tile([P, HH], BF16, tag=f"te{h}")
                nc.vector.tensor_tensor(te[:], qe[:, hs], h_e[:, hs],
                                        op=OP.mult)
                nc.vector.tensor_scalar(ge[:, gsl], te[:], -0.9, 1.0,
                                        op0=OP.max, op1=OP.add)
                tr = scr.tile([P, HH], BF16, tag=f"tr{h}")
                nc.gpsimd.tensor_tensor(tr[:], qr[:, hs], h_r[:, hs],
                                        op=OP.mult)
                nc.gpsimd.tensor_scalar(gr[:, gsl], tr[:], 0.1, 0.01,
                                        op0=OP.add, op1=OP.max)

            def post_halos(s):
                qe, qr, ge, gr, se, sr_, v = post_ctx[s]
                for g_t in (ge, gr):
                    # half1 warmup <- tail of half0 (same partition: engine
                    # copy); half0 <- tail of previous partition's half1 (DMA)
                    nc.vector.tensor_copy(g_t[:, AH:AH + W],
                                          g_t[:, AH - W:AH])
                    nc.sync.dma_start(g_t[1:P, 0:W], g_t[0:P - 1, 2 * AH - W:])
                    nc.gpsimd.memset(g_t[0:1, 0:W], 0.0)
                    # stream start: state must jump to 10*g0
                    nc.scalar.mul(g_t[0:1, W:W + 1], g_t[0:1, W:W + 1], 10.0)

            def post_scan_half(s, h):
                qe, qr, ge, gr, se, sr_, v = post_ctx[s]
                hsl = slice(AH * h, AH * (h + 1))
                nc.vector.tensor_tensor_scan(
                    se[:, hsl], d0[:], ge[:, hsl], 0.0,
                    op0=OP.mult, op1=OP.add)
                nc.vector.tensor_tensor_scan(
                    sr_[:, hsl], d0[:], gr[:, hsl], 0.0,
                    op0=OP.mult, op1=OP.add)

            def post_mix_half(s, h):
                qe, qr, ge, gr, se, sr_, v = post_ctx[s]
                enh_t, res_t = audio_sr[s]
                hs = slice(HH * h, HH * (h + 1))
                gsl = slice(AH * h + W, AH * (h + 1))
                c1 = scr.tile([P, HH], BF16, tag=f"c1{h}")
                nc.vector.tensor_tensor(c1[:], enh_t[:, hs], se[:, gsl],
                                        op=OP.mult)
                c2 = scr.tile([P, HH], BF16, tag=f"c2{h}")
                nc.gpsimd.tensor_tensor(c2[:], res_t[:, hs], sr_[:, gsl],
                                        op=OP.mult)
                nc.vector.tensor_tensor(v[:, hs], c1[:], c2[:], op=OP.add)
                nc.vector.tensor_reduce(
                    vm[:, 2 * s + h:2 * s + h + 1], v[:, hs],
                    op=OP.max, axis=mybir.AxisListType.X,
                    apply_absolute_value=True)

            def post_all(s):
                post_stage0(s)
                post_stage1(s)
                post_gain_half(s, 0)
                post_gain_half(s, 1)
                post_halos(s)
                post_scan_half(s, 0)
                post_scan_half(s, 1)
                post_mix_half(s, 0)
                post_mix_half(s, 1)

            # ---- MLP: 16 chunks; bf16 matmuls; psum->sbuf copies via DMA ----
            for c in range(NCHK):
                xac = xa[:, c * CW:(c + 1) * CW]
                pe_touch(xac)
                xps = ps.tile([128, 8 * 128], BF16, tag="xps")
                for j in range(CHK // TT):
                    nc.tensor.transpose(
                        xps[:, j * 128:j * 128 + TT],
                        xac[:, j * GRU_H:j * GRU_H + 128], identb[:TT, :TT])
                    nc.tensor.transpose(
                        xps[:, 512 + j * 128:512 + j * 128 + TT],
                        xac[:, j * GRU_H + 128:j * GRU_H + 256],
                        identb[:TT, :TT])
                # full-width PSUM->SBUF copy (bf16; packed cols so the bf16
                # fast path applies); matmuls read 125-of-128 col subtiles.
                xr = mlp.tile([128, 8 * 128], BF16, tag="xr")
                if c < 8:
                    nc.vector.tensor_copy(xr[:], xps[:])
                else:
                    nc.scalar.copy(xr[:], xps[:])

                yps = ps.tile([128, CHK], F32, tag="yz")
                pe_touch(xr)
                for j in range(CHK // TT):
                    x0j = xr[:, j * 128:j * 128 + TT]
                    x1j = xr[:, 512 + j * 128:512 + j * 128 + TT]
                    yj = yps[:, j * TT:(j + 1) * TT]
                    nc.tensor.matmul(yj, wsr["w1t0"][:], x0j,
                                     start=True, stop=False)
                    nc.tensor.matmul(yj, wsr["w1t1"][:], x1j,
                                     start=False, stop=True)
                ay = mlp.tile([128, CHK], BF16, tag="ay")
                nc.scalar.activation(ay[:], yps[:], AF.Abs, bias=wsb["bias1"][:])

                zfull = ps.tile([128, CHK], F32, tag="yz")
                zps = zfull[0:65, :]
                for j in range(CHK // TT):
                    x0j = xr[:, j * 128:j * 128 + TT]
                    x1j = xr[:, 512 + j * 128:512 + j * 128 + TT]
                    zj = zps[:, j * TT:(j + 1) * TT]
                    nc.tensor.matmul(zj, wsr["a2xt0"][:], x0j,
                                     start=True, stop=False)
                    nc.tensor.matmul(zj, wsr["a2xt1"][:], x1j,
                                     start=False, stop=False)
                    nc.tensor.matmul(zj, wsr["b2xt"][:],
                                     ay[:, j * TT:(j + 1) * TT],
                                     start=False, stop=True)
                t2 = mlp.tile([65, CHK], F32, tag="t2")
                nc.scalar.activation(t2[:], zps[:], AF.Abs, bias=wsb["bias2"][:])

                for j in range(CHK // TT):
                    cc = c * (CHK // TT) + j
                    nc.tensor.matmul(p2ps[:, cc:cc + 1],
                                     t2[:, j * TT:(j + 1) * TT], wsb["r3"][:],
                                     start=True, stop=True)
                if c == 9:
                    audio_pre(1)
                if c == 7:
                    post_stage0(0)
                elif c == 8:
                    post_stage1(0)
                elif c == 9:
                    post_gain_half(0, 0)
                    post_gain_half(0, 1)
                elif c == 10:
                    post_halos(0)
                elif c == 11:
                    post_scan_half(0, 0)
                elif c == 12:
                    post_scan_half(0, 1)
                    post_mix_half(0, 0)
                elif c == 13:
                    post_mix_half(0, 1)
                elif c == NCHK - 1:
                    post_all(1)

            # ---- global normalization ----
            gmax = small.tile([P, 2], F32, tag="gmax")
            nc.vector.tensor_reduce(gmax[:, 0:1], vm[:], op=OP.max,
                                    axis=mybir.AxisListType.X)
            nc.vector.tensor_reduce(gmax[:, 1:2], em[:], op=OP.max,
                                    axis=mybir.AxisListType.X)
            gmr = small.tile([P, 2], F32, tag="gmr")
            nc.gpsimd.partition_all_reduce(gmr[:], gmax[:], channels=P,
                                           reduce_op=bass_isa.ReduceOp.max)
            ccsb = small.tile([1, 2 * NCORES], F32, tag="ccsb")
            nvis = 2 if sim else NCORES
            if sim:
                for cpy in range(nvis):
                    nc.sync.dma_start(ccsb[0:1, 2 * cpy:2 * cpy + 2],
                                      gmr[0:1, 0:2])
            else:
                with tc.tile_critical():
                    cc_sem = nc.alloc_semaphore("ccs")
                    nc.gpsimd.dma_start(cc_in[:], gmr[0:1, 0:2]).then_inc(
                        cc_sem, 16)
                    nc.gpsimd.collective_compute(
                        "AllGather", OP.bypass,
                        replica_groups=[list(range(NCORES))],
                        ins=[cc_in[:]], outs=[cc_out[:]],
                    )._wait_ge(cc_sem, 16).then_inc(cc_sem, 1)
                    nc.gpsimd.dma_start(ccsb[:], cc_out[None, :])._wait_ge(
                        cc_sem, 17).then_inc(cc_sem, 16)
                    nc.gpsimd.engine_nop()._wait_ge(cc_sem, 33)

            sg = small.tile([1, 4], F32, tag="sg")
            nc.vector.tensor_reduce(sg[:, 0:1], ccsb[:, 0:2 * nvis:2],
                                    op=OP.max, axis=mybir.AxisListType.X)
            nc.vector.tensor_reduce(sg[:, 1:2], ccsb[:, 1:2 * nvis:2],
                                    op=OP.max, axis=mybir.AxisListType.X)
            # sigma = emax / (vmax + 1e-7)   (v = 10*out_unnorm folds alpha)
            nc.vector.tensor_scalar(sg[:, 2:3], sg[:, 0:1], 1e-7, None,
                                    op0=OP.add)
            nc.vector.reciprocal_approx_fast(out=sg[:, 0:1], in_=sg[:, 2:3])
            nc.vector.tensor_tensor(sg[:, 3:4], sg[:, 0:1], sg[:, 1:2],
                                    op=OP.mult)
            sgb = small.tile([P, 1], F32, tag="sgb")
            nc.gpsimd.partition_broadcast(sgb[:], sg[0:1, 3:4], channels=P)

            for s in range(S):
                oview = out[s].rearrange("(p n) -> p n", p=P)
                for h in range(2):
                    hs = slice(HH * h, HH * (h + 1))
                    of = scr.tile([P, HH], F32, tag=f"of{h}")
                    nc.vector.tensor_scalar(of[:], vout[s][:, hs],
                                            sgb[:, 0:1], None, op0=OP.mult)
                    nc.sync.dma_start(oview[:, hs], of[:])
    nc.finalize()
    return nc


def kernel(trace=False, **inputs):
    gru = np.ascontiguousarray(np.asarray(inputs["gru_output"], np.float32))
    enh = np.ascontiguousarray(np.asarray(inputs["enhanced"], np.float32))
    noisy = np.ascontiguousarray(np.asarray(inputs["noisy"], np.float32))
    B = gru.shape[0]
    wts = _prep_weights(inputs["W1"], inputs["b1"], inputs["a1"],
                        inputs["W2"], inputs["b2"], inputs["a2"],
                        inputs["W3"], inputs["b3"])
    m3 = _interp_m3()
    m3d = np.zeros((35, HOP), np.float32)
    m3d[0:3] = m3
    m3d[32:35] = m3
    wts["m3d"] = m3d
    wts["ident"] = np.ascontiguousarray(np.eye(128, dtype=np.float32))

    if "nc" not in _compiled:
        _compiled["nc"] = _build_nc()
    nc = _compiled["nc"]

    per = B // NCORES
    in_maps = []
    for c in range(NCORES):
        m = {
            "gru": np.ascontiguousarray(
                gru[c * per:(c + 1) * per].reshape(TB, GRU_H)),
            "enh": np.ascontiguousarray(enh[c * per:(c + 1) * per]),
            "noisy": np.ascontiguousarray(noisy[c * per:(c + 1) * per]),
        }
        m.update(wts)
        in_maps.append(m)

    res = run_bass_kernel_spmd(nc, in_maps, list(range(NCORES)), trace=trace)
    outs = [res.results[c]["out"] for c in range(NCORES)]
    full = np.concatenate(outs, axis=0)
    if trace:
        return full, res
    return full


if __name__ == "__main__":
    pass
